# revision 12
# baseline (speedup 1.0000x reference)
"""CapsuleLayer forward (squash + per-capsule matmul) on 8 Trainium2 cores.

Reference computation (all fp32):
    x  = inputs.reshape(B, 1152, 8)
    pc = squash(x)                              # per-(b,n) over k=8
    u_hat[b,n,j,d] = sum_k W[0,n,j,d,k] * pc[b,n,k]
    out = u_hat[..., None]                      # [B, 1152, 10, 16, 1]

Sharding: capsule dim (n=1152) split 144-per-core across 8 cores; every core
keeps the full batch (B=512).  Zero cross-device communication.

Per-core kernel (all fp16 data paths; PSUM accumulates fp32):
  - x host-cast to fp16; squash computed in fp16 (one fused reciprocal);
    next chunk's squash is emitted mid-chunk so the chain hides under evacs
  - W host-packed into FULL 16-cap block-diagonal [128, 2560] fp16 tiles,
    one clean DMA per group into 9 resident SBUF tiles
  - pc transposed to [ck, b] via PE transpose (fp16 identity), pipelined one
    group ahead so the PE never waits on the PSUM->SBUF pcT copy
  - matmul out[b, (c,jd)] = pcT.T @ wblk  (K=128, M=128, fp16 operands)
  - PSUM->SBUF evacuation in [128,1024] chunks split DVE/ACT; output
    accumulated into 3-group [128, 7680] fp16 tiles and stored with 12
    large DMAs alternating between the sync and scalar rings
"""

from contextlib import ExitStack

import numpy as np

import concourse.bacc as bacc
import concourse.bass as bass  # noqa: F401  (AP helpers)
import concourse.mybir as mybir
import concourse.tile as tile
from concourse.bass_utils import run_bass_kernel_spmd
from concourse.masks import make_identity

N_CORES = 8
B = 512
N_CAPS = 1152
K = 8
JD = 160  # 10*16
CAPS_PER_CORE = N_CAPS // N_CORES  # 144
GROUP_CAPS = 16  # caps per matmul group -> K=128
N_GROUPS = CAPS_PER_CORE // GROUP_CAPS  # 9
GROUP_COLS = GROUP_CAPS * JD  # 2560
P = 128
B_CHUNKS = B // P  # 4
EPS = 1e-07
SG = 3  # groups per output store
SG_COLS = SG * GROUP_COLS  # 7680

F32 = mybir.dt.float32
F16 = mybir.dt.float16
OUT_DT = mybir.dt.float16
OUT_NP = np.float16


def build_program():
    nc = bacc.Bacc("TRN2", debug=False, num_devices=N_CORES)
    x = nc.dram_tensor("x", [B, CAPS_PER_CORE * K], F16, kind="ExternalInput").ap()
    wt = nc.dram_tensor(
        "wt", [CAPS_PER_CORE * K, GROUP_COLS], F16, kind="ExternalInput"
    ).ap()
    out = nc.dram_tensor(
        "out", [B, CAPS_PER_CORE * JD], OUT_DT, kind="ExternalOutput"
    ).ap()

    with tile.TileContext(nc) as tc, ExitStack() as ctx:
        consts = ctx.enter_context(tc.tile_pool(name="consts", bufs=1))
        wblk_pool = ctx.enter_context(tc.tile_pool(name="wblk", bufs=1))
        xpool = ctx.enter_context(tc.tile_pool(name="xpool", bufs=2))
        pcpool = ctx.enter_context(tc.tile_pool(name="pcpool", bufs=2))
        stats = ctx.enter_context(tc.tile_pool(name="stats", bufs=2))
        pct_pool = ctx.enter_context(tc.tile_pool(name="pct", bufs=3))
        ost_pool = ctx.enter_context(tc.tile_pool(name="ost", bufs=2))
        # PSUM: 3x 2-bank matmul slots + 2x 1-bank transpose slots = 8 banks.
        psum = ctx.enter_context(tc.tile_pool(name="psum", bufs=3, space="PSUM"))
        psum_t = ctx.enter_context(tc.tile_pool(name="psum_t", bufs=2, space="PSUM"))

        # Chunk-0 x load goes out first so squash can start immediately;
        # the 9 weight-tile DMAs stream in behind it on the same ring.
        xt0 = xpool.tile([P, CAPS_PER_CORE, K], F16, tag="xt")
        nc.scalar.dma_start(
            out=xt0, in_=x[0:P, :].rearrange("b (c k) -> b c k", k=K)
        )

        identity = consts.tile([P, P], F16)
        make_identity(nc, identity)
        eps_tile = consts.tile([P, 1], F32)
        nc.vector.memset(eps_tile, EPS)

        wblk = []
        for g in range(N_GROUPS):
            wb = wblk_pool.tile([P, GROUP_COLS], F16, tag=f"wblk{g}")
            nc.scalar.dma_start(out=wb, in_=wt[g * P : (g + 1) * P, :])
            wblk.append(wb)

        def emit_squash(bi, xt):
            # squash: scale[b,c] = sq / ((1+sq)*sqrt(sq+eps)), pc = x*scale
            x2 = xpool.tile([P, CAPS_PER_CORE, K], F16)
            nc.vector.tensor_mul(x2, xt, xt)
            sq = stats.tile([P, CAPS_PER_CORE], F16)
            nc.vector.reduce_sum(out=sq, in_=x2, axis=mybir.AxisListType.X)
            sn = stats.tile([P, CAPS_PER_CORE], F16)
            nc.scalar.activation(
                out=sn, in_=sq, func=mybir.ActivationFunctionType.Sqrt,
                bias=eps_tile, scale=1.0,
            )
            t1 = stats.tile([P, CAPS_PER_CORE], F16)
            nc.scalar.add(t1, sq, 1.0)
            den = stats.tile([P, CAPS_PER_CORE], F16)
            nc.vector.tensor_mul(den, t1, sn)
            rden = stats.tile([P, CAPS_PER_CORE], F16)
            nc.vector.reciprocal(rden, den)
            scale = stats.tile([P, CAPS_PER_CORE], F16)
            nc.vector.tensor_mul(scale, sq, rden)
            pc = pcpool.tile([P, CAPS_PER_CORE, K], F16)
            nc.vector.tensor_mul(
                pc, xt, scale.unsqueeze(2).broadcast_to([P, CAPS_PER_CORE, K])
            )
            return pc.rearrange("p c k -> p (c k)")

        def issue_transpose(pc_flat, g):
            # Pipelined one group ahead: the PSUM->SBUF copy jumps ahead of
            # the evac backlog on the DVE/ACT queues so PE never waits.
            pst = psum_t.tile([P, P], F16, tag="pt")
            nc.tensor.transpose(pst, pc_flat[:, g * P : (g + 1) * P], identity)
            pcT = pct_pool.tile([P, P], F16)
            if g % 2 == 0:
                nc.scalar.copy(pcT, pst)
            else:
                nc.vector.tensor_copy(pcT, pst)
            return pcT

        with nc.allow_low_precision("fp16 squash: tolerance is 2e-2"):
            pc_cur = emit_squash(0, xt0)
            pc_next = None
            for bi in range(B_CHUNKS):
                pcT_next = issue_transpose(pc_cur, 0)
                ost = None
                for g in range(N_GROUPS):
                    pcT = pcT_next
                    if g + 1 < N_GROUPS:
                        pcT_next = issue_transpose(pc_cur, g + 1)

                    if g == 2 and bi + 1 < B_CHUNKS:
                        # Prefetch + squash the next chunk mid-way through
                        # this one: the chain drains under the evac traffic.
                        xt = xpool.tile([P, CAPS_PER_CORE, K], F16, tag="xt")
                        nc.scalar.dma_start(
                            out=xt,
                            in_=x[(bi + 1) * P : (bi + 2) * P, :].rearrange(
                                "b (c k) -> b c k", k=K
                            ),
                        )
                        pc_next = emit_squash(bi + 1, xt)

                    pa = psum.tile([P, 1024], F32, tag="pm")
                    pb = psum.tile([P, 1024], F32, tag="pm")
                    pcs = psum.tile([P, 512], F32, tag="pm")
                    for s in range(2):
                        nc.tensor.matmul(
                            pa[:, s * 512 : (s + 1) * 512],
                            lhsT=pcT,
                            rhs=wblk[g][:, s * 512 : (s + 1) * 512],
                            start=True,
                            stop=True,
                        )
                    for s in range(2):
                        nc.tensor.matmul(
                            pb[:, s * 512 : (s + 1) * 512],
                            lhsT=pcT,
                            rhs=wblk[g][:, (2 + s) * 512 : (3 + s) * 512],
                            start=True,
                            stop=True,
                        )
                    nc.tensor.matmul(
                        pcs, lhsT=pcT, rhs=wblk[g][:, 4 * 512 : 5 * 512],
                        start=True, stop=True,
                    )

                    if g % SG == 0:
                        ost = ost_pool.tile([P, SG_COLS], OUT_DT)
                    o0 = (g % SG) * GROUP_COLS
                    # Balance PSUM->SBUF evacuation across DVE and ACT: each
                    # takes one 1024 chunk; the 512 tail alternates.
                    nc.vector.tensor_copy(ost[:, o0 : o0 + 1024], pa)
                    nc.scalar.copy(ost[:, o0 + 1024 : o0 + 2048], pb)
                    if g % 2 == 0:
                        nc.vector.tensor_copy(ost[:, o0 + 2048 : o0 + 2560], pcs)
                    else:
                        nc.scalar.copy(ost[:, o0 + 2048 : o0 + 2560], pcs)
                    if g % SG == SG - 1:
                        sg = g // SG
                        ring = nc.sync if (bi * SG + sg) % 2 == 0 else nc.scalar
                        ring.dma_start(
                            out=out[
                                bi * P : (bi + 1) * P,
                                sg * SG_COLS : (sg + 1) * SG_COLS,
                            ],
                            in_=ost,
                        )
                pc_cur = pc_next
    nc.compile()
    return nc


_PROGRAM = None


def _get_program():
    global _PROGRAM
    if _PROGRAM is None:
        _PROGRAM = build_program()
    return _PROGRAM


def shard_inputs(inputs: np.ndarray, W: np.ndarray) -> list[dict[str, np.ndarray]]:
    # W -> k-major [n, k, jd], packed as FULL 16-cap block-diagonal
    # [128, 2560] fp16 tiles: row (c,k) has W[n=c] in cols [160c, 160(c+1)),
    # zeros elsewhere.  One DMA per group straight into SBUF residency.
    wt_kmaj = np.asarray(W[0], dtype=np.float32).reshape(N_CAPS, JD, K)
    wt_kmaj = wt_kmaj.transpose(0, 2, 1)  # [n, k, jd]
    grp = wt_kmaj.reshape(N_CAPS // GROUP_CAPS, GROUP_CAPS, K, JD)
    wtb = np.zeros(
        (N_CAPS // GROUP_CAPS, GROUP_CAPS, K, GROUP_CAPS, JD), dtype=np.float16
    )
    for ci in range(GROUP_CAPS):
        wtb[:, ci, :, ci, :] = grp[:, ci]
    wtb = wtb.reshape(N_CAPS * K, GROUP_COLS)
    x16 = np.asarray(inputs, dtype=np.float16)
    in_maps = []
    for i in range(N_CORES):
        c0 = i * CAPS_PER_CORE
        in_maps.append(
            {
                "x": np.ascontiguousarray(
                    x16[:, c0 * K : (c0 + CAPS_PER_CORE) * K]
                ),
                "wt": np.ascontiguousarray(
                    wtb[c0 * K : (c0 + CAPS_PER_CORE) * K]
                ),
            }
        )
    return in_maps


def unshard_output(results: list[dict[str, np.ndarray]]) -> np.ndarray:
    full = np.empty((B, N_CAPS, JD), dtype=np.float32)
    for i in range(N_CORES):
        c0 = i * CAPS_PER_CORE
        full[:, c0 : c0 + CAPS_PER_CORE, :] = results[i]["out"].reshape(
            B, CAPS_PER_CORE, JD
        ).astype(np.float32)
    return full.reshape(B, N_CAPS, 10, 16, 1)


def kernel(inputs: np.ndarray, W: np.ndarray) -> np.ndarray:
    nc = _get_program()
    in_maps = shard_inputs(np.asarray(inputs), np.asarray(W))
    res = run_bass_kernel_spmd(nc, in_maps, core_ids=list(range(N_CORES)))
    return unshard_output(res.results)


# revision 13
# speedup vs baseline: 1.0480x; 1.0480x over previous
"""CapsuleLayer forward (squash + per-capsule matmul) on 8 Trainium2 cores.

Reference computation (all fp32):
    x  = inputs.reshape(B, 1152, 8)
    pc = squash(x)                              # per-(b,n) over k=8
    u_hat[b,n,j,d] = sum_k W[0,n,j,d,k] * pc[b,n,k]
    out = u_hat[..., None]                      # [B, 1152, 10, 16, 1]

Sharding: capsule dim (n=1152) split 144-per-core across 8 cores; every core
keeps the full batch (B=512).  Zero cross-device communication.

Per-core kernel (all fp16 data paths; PSUM accumulates fp32):
  - x host-cast to fp16; squash computed in fp16 (one fused reciprocal);
    next chunk's squash is emitted mid-chunk so the chain hides under evacs
  - W host-packed into FULL 16-cap block-diagonal [128, 2560] fp16 tiles,
    one clean DMA per group into 9 resident SBUF tiles
  - pc transposed to [ck, b] via PE transpose (fp16 identity), pipelined one
    group ahead so the PE never waits on the PSUM->SBUF pcT copy
  - matmul out[b, (c,jd)] = pcT.T @ wblk  (K=128, M=128, fp16 operands)
  - PSUM->SBUF evacuation in [128,1024] chunks split DVE/ACT; output
    accumulated into 3-group [128, 7680] fp16 tiles and stored with 12
    large DMAs alternating between the sync and scalar rings
"""

from contextlib import ExitStack

import numpy as np

import concourse.bacc as bacc
import concourse.bass as bass  # noqa: F401  (AP helpers)
import concourse.mybir as mybir
import concourse.tile as tile
from concourse.bass_utils import run_bass_kernel_spmd
from concourse.masks import make_identity

N_CORES = 8
B = 512
N_CAPS = 1152
K = 8
JD = 160  # 10*16
CAPS_PER_CORE = N_CAPS // N_CORES  # 144
GROUP_CAPS = 16  # caps per matmul group -> K=128
N_GROUPS = CAPS_PER_CORE // GROUP_CAPS  # 9
GROUP_COLS = GROUP_CAPS * JD  # 2560
P = 128
B_CHUNKS = B // P  # 4
EPS = 1e-07
SG = 3  # groups per output store
SG_COLS = SG * GROUP_COLS  # 7680

F32 = mybir.dt.float32
F16 = mybir.dt.float16
OUT_DT = mybir.dt.float16
OUT_NP = np.float16


def build_program():
    nc = bacc.Bacc("TRN2", debug=False, num_devices=N_CORES)
    x = nc.dram_tensor("x", [B, CAPS_PER_CORE * K], F16, kind="ExternalInput").ap()
    wt = nc.dram_tensor(
        "wt", [CAPS_PER_CORE * K, GROUP_COLS], F16, kind="ExternalInput"
    ).ap()
    out = nc.dram_tensor(
        "out", [B, CAPS_PER_CORE * JD], OUT_DT, kind="ExternalOutput"
    ).ap()

    with tile.TileContext(nc) as tc, ExitStack() as ctx:
        consts = ctx.enter_context(tc.tile_pool(name="consts", bufs=1))
        wblk_pool = ctx.enter_context(tc.tile_pool(name="wblk", bufs=1))
        xpool = ctx.enter_context(tc.tile_pool(name="xpool", bufs=2))
        pcpool = ctx.enter_context(tc.tile_pool(name="pcpool", bufs=2))
        stats = ctx.enter_context(tc.tile_pool(name="stats", bufs=2))
        pct_pool = ctx.enter_context(tc.tile_pool(name="pct", bufs=3))
        ost_pool = ctx.enter_context(tc.tile_pool(name="ost", bufs=3))
        # PSUM: 3x 2-bank matmul slots + 2x 1-bank transpose slots = 8 banks.
        psum = ctx.enter_context(tc.tile_pool(name="psum", bufs=3, space="PSUM"))
        psum_t = ctx.enter_context(tc.tile_pool(name="psum_t", bufs=2, space="PSUM"))

        # Chunk-0 x load goes out first so squash can start immediately;
        # the 9 weight-tile DMAs stream in behind it on the same ring.
        xt0 = xpool.tile([P, CAPS_PER_CORE, K], F16, tag="xt")
        nc.scalar.dma_start(
            out=xt0, in_=x[0:P, :].rearrange("b (c k) -> b c k", k=K)
        )

        identity = consts.tile([P, P], F16)
        make_identity(nc, identity)
        eps_tile = consts.tile([P, 1], F32)
        nc.vector.memset(eps_tile, EPS)

        wblk = []
        for g in range(N_GROUPS):
            wb = wblk_pool.tile([P, GROUP_COLS], F16, tag=f"wblk{g}")
            nc.scalar.dma_start(out=wb, in_=wt[g * P : (g + 1) * P, :])
            wblk.append(wb)

        def emit_squash(bi, xt):
            # squash: scale[b,c] = sq / ((1+sq)*sqrt(sq+eps)), pc = x*scale
            x2 = xpool.tile([P, CAPS_PER_CORE, K], F16)
            nc.vector.tensor_mul(x2, xt, xt)
            sq = stats.tile([P, CAPS_PER_CORE], F16)
            nc.vector.reduce_sum(out=sq, in_=x2, axis=mybir.AxisListType.X)
            sn = stats.tile([P, CAPS_PER_CORE], F16)
            nc.scalar.activation(
                out=sn, in_=sq, func=mybir.ActivationFunctionType.Sqrt,
                bias=eps_tile, scale=1.0,
            )
            t1 = stats.tile([P, CAPS_PER_CORE], F16)
            nc.scalar.add(t1, sq, 1.0)
            den = stats.tile([P, CAPS_PER_CORE], F16)
            nc.vector.tensor_mul(den, t1, sn)
            rden = stats.tile([P, CAPS_PER_CORE], F16)
            nc.vector.reciprocal(rden, den)
            scale = stats.tile([P, CAPS_PER_CORE], F16)
            nc.vector.tensor_mul(scale, sq, rden)
            pc = pcpool.tile([P, CAPS_PER_CORE, K], F16)
            nc.vector.tensor_mul(
                pc, xt, scale.unsqueeze(2).broadcast_to([P, CAPS_PER_CORE, K])
            )
            return pc.rearrange("p c k -> p (c k)")

        def issue_transpose(pc_flat, g):
            # Pipelined one group ahead: the PSUM->SBUF copy jumps ahead of
            # the evac backlog on the DVE/ACT queues so PE never waits.
            pst = psum_t.tile([P, P], F16, tag="pt")
            nc.tensor.transpose(pst, pc_flat[:, g * P : (g + 1) * P], identity)
            pcT = pct_pool.tile([P, P], F16)
            if g % 2 == 0:
                nc.scalar.copy(pcT, pst)
            else:
                nc.vector.tensor_copy(pcT, pst)
            return pcT

        with nc.allow_low_precision("fp16 squash: tolerance is 2e-2"):
            pc_cur = emit_squash(0, xt0)
            pc_next = None
            for bi in range(B_CHUNKS):
                pcT_next = issue_transpose(pc_cur, 0)
                ost = None
                for g in range(N_GROUPS):
                    pcT = pcT_next
                    if g + 1 < N_GROUPS:
                        pcT_next = issue_transpose(pc_cur, g + 1)

                    if g == 2 and bi + 1 < B_CHUNKS:
                        # Prefetch + squash the next chunk mid-way through
                        # this one: the chain drains under the evac traffic.
                        xt = xpool.tile([P, CAPS_PER_CORE, K], F16, tag="xt")
                        nc.scalar.dma_start(
                            out=xt,
                            in_=x[(bi + 1) * P : (bi + 2) * P, :].rearrange(
                                "b (c k) -> b c k", k=K
                            ),
                        )
                        pc_next = emit_squash(bi + 1, xt)

                    pa = psum.tile([P, 1024], F32, tag="pm")
                    pb = psum.tile([P, 1024], F32, tag="pm")
                    pcs = psum.tile([P, 512], F32, tag="pm")
                    for s in range(2):
                        nc.tensor.matmul(
                            pa[:, s * 512 : (s + 1) * 512],
                            lhsT=pcT,
                            rhs=wblk[g][:, s * 512 : (s + 1) * 512],
                            start=True,
                            stop=True,
                        )
                    for s in range(2):
                        nc.tensor.matmul(
                            pb[:, s * 512 : (s + 1) * 512],
                            lhsT=pcT,
                            rhs=wblk[g][:, (2 + s) * 512 : (3 + s) * 512],
                            start=True,
                            stop=True,
                        )
                    nc.tensor.matmul(
                        pcs, lhsT=pcT, rhs=wblk[g][:, 4 * 512 : 5 * 512],
                        start=True, stop=True,
                    )

                    if g % SG == 0:
                        ost = ost_pool.tile([P, SG_COLS], OUT_DT)
                    o0 = (g % SG) * GROUP_COLS
                    # Balance PSUM->SBUF evacuation across DVE and ACT: each
                    # takes one 1024 chunk; the 512 tail alternates.
                    nc.vector.tensor_copy(ost[:, o0 : o0 + 1024], pa)
                    nc.scalar.copy(ost[:, o0 + 1024 : o0 + 2048], pb)
                    if g % 2 == 0:
                        nc.vector.tensor_copy(ost[:, o0 + 2048 : o0 + 2560], pcs)
                    else:
                        nc.scalar.copy(ost[:, o0 + 2048 : o0 + 2560], pcs)
                    if g % SG == SG - 1:
                        sg = g // SG
                        ring = nc.sync if (bi * SG + sg) % 2 == 0 else nc.scalar
                        ring.dma_start(
                            out=out[
                                bi * P : (bi + 1) * P,
                                sg * SG_COLS : (sg + 1) * SG_COLS,
                            ],
                            in_=ost,
                        )
                pc_cur = pc_next
    nc.compile()
    return nc


_PROGRAM = None


def _get_program():
    global _PROGRAM
    if _PROGRAM is None:
        _PROGRAM = build_program()
    return _PROGRAM


def shard_inputs(inputs: np.ndarray, W: np.ndarray) -> list[dict[str, np.ndarray]]:
    # W -> k-major [n, k, jd], packed as FULL 16-cap block-diagonal
    # [128, 2560] fp16 tiles: row (c,k) has W[n=c] in cols [160c, 160(c+1)),
    # zeros elsewhere.  One DMA per group straight into SBUF residency.
    wt_kmaj = np.asarray(W[0], dtype=np.float32).reshape(N_CAPS, JD, K)
    wt_kmaj = wt_kmaj.transpose(0, 2, 1)  # [n, k, jd]
    grp = wt_kmaj.reshape(N_CAPS // GROUP_CAPS, GROUP_CAPS, K, JD)
    wtb = np.zeros(
        (N_CAPS // GROUP_CAPS, GROUP_CAPS, K, GROUP_CAPS, JD), dtype=np.float16
    )
    for ci in range(GROUP_CAPS):
        wtb[:, ci, :, ci, :] = grp[:, ci]
    wtb = wtb.reshape(N_CAPS * K, GROUP_COLS)
    x16 = np.asarray(inputs, dtype=np.float16)
    in_maps = []
    for i in range(N_CORES):
        c0 = i * CAPS_PER_CORE
        in_maps.append(
            {
                "x": np.ascontiguousarray(
                    x16[:, c0 * K : (c0 + CAPS_PER_CORE) * K]
                ),
                "wt": np.ascontiguousarray(
                    wtb[c0 * K : (c0 + CAPS_PER_CORE) * K]
                ),
            }
        )
    return in_maps


def unshard_output(results: list[dict[str, np.ndarray]]) -> np.ndarray:
    full = np.empty((B, N_CAPS, JD), dtype=np.float32)
    for i in range(N_CORES):
        c0 = i * CAPS_PER_CORE
        full[:, c0 : c0 + CAPS_PER_CORE, :] = results[i]["out"].reshape(
            B, CAPS_PER_CORE, JD
        ).astype(np.float32)
    return full.reshape(B, N_CAPS, 10, 16, 1)


def kernel(inputs: np.ndarray, W: np.ndarray) -> np.ndarray:
    nc = _get_program()
    in_maps = shard_inputs(np.asarray(inputs), np.asarray(W))
    res = run_bass_kernel_spmd(nc, in_maps, core_ids=list(range(N_CORES)))
    return unshard_output(res.results)
